# revision 1
# baseline (speedup 1.0000x reference)
"""Trainium2 Bass kernel for nn_ConcatenationAggregator.

For each review r:
    out[r] = relu(concat(review_vecs[r],
                         user_vecs[adj_u[r]][perm_u],
                         item_vecs[adj_i[r]][perm_i]) @ W)

Strategy (pure data-parallel over reviews, 8 NeuronCores):
  - Feature permutations are folded into W on the host (192x64 constant).
  - Row gathers use the GPSIMD `dma_gather` ucode (int16 indices, <=1024
    indices per call).  Since the tables exceed 32768 rows, the host sorts
    each core's reviews into 8 groups by (user-table 32K chunk, item-table
    32K chunk) so that rebased indices fit int16.  The group sort is just a
    relabeling of which review each (partition, column) slot processes; the
    host un-permutes the output.
  - The review stream is host-transposed into a feature-major, packed
    128-partition layout so it feeds the PE rhs directly; the output is
    produced transposed from PSUM and unpacked on the host.
  - Gathered rows are PE-transposed (user sub-tile -> PSUM partitions 0:64,
    item -> 64:128) giving a K=128 stacked rhs so one matmul covers the
    user+item contribution; a second K=64 matmul adds the review term.
  - This toolchain build enforces ONE sync-wait slot per instruction, so
    the emission order is software-pipelined (matmuls of chunk t before the
    transposes of chunk t+1, relus of chunk t after the copies of chunk
    t+1), discarded "header" transposes absorb gather-DMA waits, and the
    kernel-tail drain is split into single-wait drains.
"""

import os
import types

import numpy as np

import concourse.bacc as bacc
import concourse.bass as bass
import concourse.mybir as mybir
import concourse.tile as tile
from concourse.bass_utils import run_bass_kernel_spmd
from concourse.masks import make_identity
from concourse.vector_clock import ScopedClock, VectorClock

F32 = mybir.dt.float32
I16 = mybir.dt.int16

N_CORES = 8
D = 64
SUB = 128                  # reviews per sub-tile
MAX_S = 8                  # sub-tiles per chunk (<=1024 gather indices)
TCH = 32768                # table chunk (int16 index range)

N_REVIEWS = 1_000_000
N_USERS = 100_000
N_ITEMS = 50_000
RPC = N_REVIEWS // N_CORES


def _split_drain_and_barrier(self, tick_clock, wait_clock):
    """Replacement for TileContext._drain_and_barrier: the stock tail drain
    waits on every live proc semaphore at once, which overflows this
    toolchain's one-sync-wait-per-instruction limit.  Emit one drain per
    semaphore instead."""
    gc = tick_clock.global_clock
    ticks = list(gc)
    idxs = [i for i, t in enumerate(ticks) if t > 0]
    for i in idxs:
        sub = [0] * len(ticks)
        sub[i] = ticks[i]
        drain_inst = self.nc.sync.drain()
        wait_clock.add_sem_waits(
            drain_inst.ins, ScopedClock({None: VectorClock(sub)}))
    if not idxs:
        drain_inst = self.nc.sync.drain()
        wait_clock.add_sem_waits(
            drain_inst.ins, ScopedClock({None: VectorClock(ticks)}))
    self.nc.all_engine_barrier()
    assert self.sems is not None
    popped = self.nc._tile_sem_poison_stack.pop()
    assert popped is self._sem_poison
    self.nc.clear_and_free_semaphores(list(self.sems.allocated().values()))
    self.nc.all_engine_barrier()


def _chunk_list(s_per_group):
    """[(group, s_subtiles, row_base_slots, idxcol_base), ...] — shared by
    host packing and device program.  s values are even, <= MAX_S."""
    chunks = []
    row = 0
    col = 0
    for g, sg in enumerate(s_per_group):
        left = sg
        while left > 0:
            s = min(MAX_S, left)
            chunks.append((g, s, row, col))
            row += s * SUB
            col += s * 8
            left -= s
    return chunks


BUFS = int(os.environ.get("KBUFS", "3"))
PREF = int(os.environ.get("KPREF", "1"))


def _build_program(chunks, n_users, n_items):
    nc = bacc.Bacc("TRN2", target_bir_lowering=False, debug=False,
                   enable_asserts=False)
    padtot = sum(s for (_, s, _, _) in chunks) * SUB
    icols = padtot // 16

    rt_d = nc.dram_tensor("rt", [64, padtot], F32, kind="ExternalInput")
    uidx_d = nc.dram_tensor("uidx", [128, icols], I16, kind="ExternalInput")
    iidx_d = nc.dram_tensor("iidx", [128, icols], I16, kind="ExternalInput")
    tblu_d = nc.dram_tensor("tblu", [n_users, D], F32, kind="ExternalInput")
    tbli_d = nc.dram_tensor("tbli", [n_items, D], F32, kind="ExternalInput")
    w1_d = nc.dram_tensor("w1", [64, 64], F32, kind="ExternalInput")
    w2_d = nc.dram_tensor("w2p", [64, 64], F32, kind="ExternalInput")
    w3_d = nc.dram_tensor("w3p", [64, 64], F32, kind="ExternalInput")
    out_d = nc.dram_tensor("out2", [64, padtot], F32, kind="ExternalOutput")

    T = len(chunks)

    with tile.TileContext(nc) as tc:
        tc._drain_and_barrier = types.MethodType(_split_drain_and_barrier, tc)
        with tc.tile_pool(name="const", bufs=1) as constp, \
             tc.tile_pool(name="rtp", bufs=BUFS) as rtp, \
             tc.tile_pool(name="gup", bufs=BUFS) as gup, \
             tc.tile_pool(name="gip", bufs=BUFS) as gip, \
             tc.tile_pool(name="outp", bufs=BUFS) as outp, \
             tc.tile_pool(name="utp", bufs=BUFS) as utp, \
             tc.tile_pool(name="itp", bufs=BUFS) as itp, \
             tc.tile_pool(name="scr", bufs=1, space="PSUM") as scrp, \
             tc.tile_pool(name="tpp", bufs=4, space="PSUM") as tpp, \
             tc.tile_pool(name="mmp", bufs=3, space="PSUM") as mmp:

            ident = constp.tile([128, 128], F32)
            make_identity(nc, ident[:])
            w1_t = constp.tile([64, 64], F32)
            nc.sync.dma_start(out=w1_t[:], in_=w1_d.ap()[:, :])
            w2_t = constp.tile([64, 64], F32)
            nc.sync.dma_start(out=w2_t[:], in_=w2_d.ap()[:, :])
            w3_t = constp.tile([64, 64], F32)
            nc.sync.dma_start(out=w3_t[:], in_=w3_d.ap()[:, :])
            uidx_t = constp.tile([128, icols], I16)
            nc.sync.dma_start(out=uidx_t[:], in_=uidx_d.ap()[:, :])
            iidx_t = constp.tile([128, icols], I16)
            nc.sync.dma_start(out=iidx_t[:], in_=iidx_d.ap()[:, :])
            dummy_sb = constp.tile([64, 128], F32)
            pscr = constp.tile([16, 16], I16)

            scratch = scrp.tile([64, 128], F32)
            # PE warmups: observe the identity (Pool) and weight-load (HWDGE)
            # semaphores with one wait each.
            nc.tensor.transpose(out=scratch[:], in_=ident[:, 0:64],
                                identity=ident[:])
            nc.tensor.matmul(out=scratch[:, 0:64], lhsT=w2_t[:],
                             rhs=w2_t[:], start=True, stop=True)
            nc.tensor.matmul(out=scratch[:, 0:64], lhsT=w3_t[:],
                             rhs=w3_t[:], start=True, stop=True)
            nc.tensor.matmul(out=scratch[:, 0:64], lhsT=w1_t[:],
                             rhs=w1_t[:], start=True, stop=True)
            # Pool warmups: observe the index-table loads.
            nc.gpsimd.tensor_copy(out=pscr[:, :], in_=uidx_t[0:16, 0:16])
            nc.gpsimd.tensor_copy(out=pscr[:, :], in_=iidx_t[0:16, 0:16])

            nreg = {}
            for (_, s, _, _) in chunks:
                if s not in nreg:
                    nreg[s] = nc.gpsimd.to_reg(s * SUB)

            ubase = [g // 2 * TCH for g in range(8)]
            usize = [min(TCH, n_users - b) for b in ubase]
            ibase = [g % 2 * TCH for g in range(8)]
            isize = [min(TCH, n_items - b) for b in ibase]

            rt_tiles = [None] * T
            gu_tiles = [None] * T
            gi_tiles = [None] * T
            ui_tiles = [None] * T
            ps_tiles = [None] * T
            o_tiles = [None] * T

            def issue_loads(t):
                g, s, row, col = chunks[t]
                rt_t = rtp.tile([64, MAX_S * 128], F32, tag="rt")
                nc.sync.dma_start(
                    out=rt_t[:, :s * 128],
                    in_=rt_d.ap()[:, row: row + s * 128])
                gu_t = gup.tile([128, MAX_S * 64], F32, tag="gu")
                nc.gpsimd.dma_gather(
                    out_ap=gu_t[:, :s * 64].rearrange("p (n d) -> p n d", d=64),
                    in_ap=tblu_d.ap()[ubase[g]:ubase[g] + usize[g], :],
                    idxs_ap=uidx_t[:, col:col + s * 8],
                    num_idxs=s * SUB, num_idxs_reg=nreg[s], elem_size=64)
                gi_t = gip.tile([128, MAX_S * 64], F32, tag="gi")
                nc.gpsimd.dma_gather(
                    out_ap=gi_t[:, :s * 64].rearrange("p (n d) -> p n d", d=64),
                    in_ap=tbli_d.ap()[ibase[g]:ibase[g] + isize[g], :],
                    idxs_ap=iidx_t[:, col:col + s * 8],
                    num_idxs=s * SUB, num_idxs_reg=nreg[s], elem_size=64)
                rt_tiles[t], gu_tiles[t], gi_tiles[t] = rt_t, gu_t, gi_t

            def issue_transposes(t):
                _, s, _, _ = chunks[t]
                gu_t, gi_t = gu_tiles[t], gi_tiles[t]
                if os.environ.get("KHDR", "0") == "1":
                    # Discarded header transposes absorb the two gather waits.
                    nc.tensor.transpose(out=scratch[:], in_=gu_t[:, 0:64],
                                        identity=ident[:])
                    nc.tensor.transpose(out=scratch[:], in_=gi_t[:, 0:64],
                                        identity=ident[:])
                ut_t = utp.tile([64, MAX_S * 128], F32, tag="ut")
                it_t = itp.tile([64, MAX_S * 128], F32, tag="it")
                for g4 in range((s + 3) // 4):
                    w = min(4, s - g4 * 4)
                    tpu = tpp.tile([64, 512], F32, tag="tp")
                    for jj in range(w):
                        j = g4 * 4 + jj
                        nc.tensor.transpose(
                            out=tpu[:, jj * 128:(jj + 1) * 128],
                            in_=gu_t[:, j * 64:(j + 1) * 64],
                            identity=ident[:])
                    nc.vector.tensor_copy(
                        out=ut_t[:, g4 * 512:g4 * 512 + w * 128],
                        in_=tpu[:, :w * 128])
                    tpi = tpp.tile([64, 512], F32, tag="tp")
                    for jj in range(w):
                        j = g4 * 4 + jj
                        nc.tensor.transpose(
                            out=tpi[:, jj * 128:(jj + 1) * 128],
                            in_=gi_t[:, j * 64:(j + 1) * 64],
                            identity=ident[:])
                    nc.vector.tensor_copy(
                        out=it_t[:, g4 * 512:g4 * 512 + w * 128],
                        in_=tpi[:, :w * 128])
                ui_tiles[t] = (ut_t, it_t)

            def issue_matmuls(t):
                _, s, _, _ = chunks[t]
                n = s * 64
                pss = []
                rt_t = rt_tiles[t]
                ut_t, it_t = ui_tiles[t]
                for q in range(2):
                    ps = mmp.tile([64, 512], F32, tag="mm")
                    ps_s = ps[:, :n]
                    nc.tensor.matmul(out=ps_s, lhsT=w2_t[:],
                                     rhs=ut_t[:, q * n:(q + 1) * n],
                                     start=True, stop=False)
                    nc.tensor.matmul(out=ps_s, lhsT=w3_t[:],
                                     rhs=it_t[:, q * n:(q + 1) * n],
                                     start=False, stop=False)
                    nc.tensor.matmul(out=ps_s, lhsT=w1_t[:],
                                     rhs=rt_t[:, q * n:(q + 1) * n],
                                     start=False, stop=True)
                    pss.append(ps)
                ps_tiles[t] = pss

            def issue_relus(t):
                _, s, _, _ = chunks[t]
                n = s * 64
                pss = ps_tiles[t]
                o_t = outp.tile([64, MAX_S * 128], F32, tag="o")
                for q in range(2):
                    nc.vector.tensor_scalar_max(
                        out=o_t[:, q * n:(q + 1) * n],
                        in0=pss[q][:, :n], scalar1=0.0)
                o_tiles[t] = o_t

            def issue_store(t):
                _, s, row, _ = chunks[t]
                nc.sync.dma_start(
                    out=out_d.ap()[:, row: row + s * 128],
                    in_=o_tiles[t][:, :s * 128])

            # Software-pipelined emission (see module docstring).
            for tt in range(min(PREF, T)):
                issue_loads(tt)
            issue_transposes(0)
            for t in range(T):
                if t + PREF < T:
                    issue_loads(t + PREF)
                issue_matmuls(t)
                if t + 1 < T:
                    issue_transposes(t + 1)
                else:
                    # Dummy PE op after the last matmuls + a DVE observer so
                    # the last relus elide their PE wait.
                    nc.tensor.transpose(out=scratch[:], in_=ident[:, 0:64],
                                        identity=ident[:])
                    nc.vector.tensor_copy(out=dummy_sb[:], in_=scratch[:])
                issue_relus(t)
                issue_store(t)
    nc.finalize()
    return nc


_PROGRAM_CACHE: dict = {}


def _get_program(chunk_key, n_users, n_items):
    key = (chunk_key, n_users, n_items)
    if key not in _PROGRAM_CACHE:
        _PROGRAM_CACHE[key] = (
            _build_program(_chunk_list(list(chunk_key)), n_users, n_items))
    return _PROGRAM_CACHE[key]


def _pack_rt(rv_sorted, chunks):
    """[PADTOT, 64] sorted/padded reviews -> [64, PADTOT] feature-major."""
    return np.ascontiguousarray(rv_sorted.T)


def _unpack_out(o2, chunks):
    """[64, PADTOT] transposed output -> [PADTOT, 64]."""
    return np.ascontiguousarray(o2.T)


def _wrap_idx(flat_sorted, chunks):
    """Rebased int16 indices [PADTOT] -> [128, PADTOT//16] in dma_gather's
    wrapped layout: per chunk block [128, 8*s] with block[p, m] =
    flat[m*16 + p%16], replicated across the 8 16-partition groups."""
    cols = []
    for (_, s, row, _) in chunks:
        blk = flat_sorted[row:row + s * SUB].reshape(s * 8, 16).T  # [16, 8s]
        cols.append(np.tile(blk, (8, 1)))
    return np.ascontiguousarray(np.concatenate(cols, axis=1))


def _run(review_vecs, user_vecs, item_vecs, W,
         review_user_adj, review_item_adj, perm_u, perm_i,
         n_cores, rpc):
    n_users = user_vecs.shape[0]
    n_items = item_vecs.shape[0]

    W = np.asarray(W, np.float32)
    W1 = np.ascontiguousarray(W[0:64])
    W2 = W[64:128]
    W3 = W[128:192]
    perm_u = np.asarray(perm_u, np.int64)
    perm_i = np.asarray(perm_i, np.int64)
    W2p = np.empty_like(W2)
    W2p[perm_u] = W2
    W3p = np.empty_like(W3)
    W3p[perm_i] = W3
    W2p = np.ascontiguousarray(W2p)
    W3p = np.ascontiguousarray(W3p)

    user_vecs = np.ascontiguousarray(np.asarray(user_vecs, np.float32))
    item_vecs = np.ascontiguousarray(np.asarray(item_vecs, np.float32))
    review_vecs = np.asarray(review_vecs, np.float32)
    au_all = np.asarray(review_user_adj, np.int64)
    ai_all = np.asarray(review_item_adj, np.int64)

    # Group each core's reviews by (user 32K chunk, item 32K chunk).
    per_core = []
    s_max = np.zeros(8, np.int64)
    for c in range(n_cores):
        lo = c * rpc
        au = au_all[lo:lo + rpc]
        ai = ai_all[lo:lo + rpc]
        grp = (au // TCH) * 2 + (ai // TCH)
        order = np.argsort(grp, kind="stable")
        counts = np.bincount(grp, minlength=8)
        per_core.append((order, counts))
        s_max = np.maximum(s_max, -(-counts // SUB))
    # shared chunk structure: even sub-tile counts per group
    s_per_group = [int(s + (s % 2)) for s in s_max]
    chunk_key = tuple(s_per_group)
    chunks = _chunk_list(s_per_group)
    padtot = sum(s for (_, s, _, _) in chunks) * SUB

    nc = _get_program(chunk_key, n_users, n_items)

    in_maps = []
    slotmaps = []
    for c in range(n_cores):
        lo = c * rpc
        au = au_all[lo:lo + rpc]
        ai = ai_all[lo:lo + rpc]
        order, counts = per_core[c]
        grp_sorted_bounds = np.cumsum(counts)
        slotmap = np.full(padtot, -1, np.int64)
        row = 0
        start = 0
        for g in range(8):
            cnt = int(counts[g])
            ids = order[start:start + cnt]
            slotmap[row:row + cnt] = ids
            start += cnt
            row += s_per_group[g] * SUB
        valid = slotmap >= 0
        sl = np.where(valid, slotmap, 0)

        rv_sorted = np.where(valid[:, None],
                             review_vecs[lo:lo + rpc][sl], 0.0).astype(np.float32)
        slot_g = np.repeat(np.arange(8), np.array(s_per_group) * SUB)
        u_reb = np.where(valid, au[sl] - (slot_g // 2) * TCH, 0).astype(np.int16)
        i_reb = np.where(valid, ai[sl] - (slot_g % 2) * TCH, 0).astype(np.int16)

        in_maps.append({
            "rt": _pack_rt(rv_sorted, chunks),
            "uidx": _wrap_idx(u_reb, chunks),
            "iidx": _wrap_idx(i_reb, chunks),
            "tblu": user_vecs,
            "tbli": item_vecs,
            "w1": W1,
            "w2p": W2p,
            "w3p": W3p,
        })
        slotmaps.append((slotmap, valid))

    res = run_bass_kernel_spmd(nc, in_maps, core_ids=list(range(n_cores)))

    out = np.empty((n_cores * rpc, 64), np.float32)
    for c in range(n_cores):
        o2 = np.asarray(res.results[c]["out2"], np.float32)
        out_sorted = _unpack_out(o2, chunks)
        slotmap, valid = slotmaps[c]
        out[c * rpc + slotmap[valid]] = out_sorted[valid]
    return out


def kernel(**inputs) -> np.ndarray:
    return _run(
        inputs["review_vecs"], inputs["user_vecs"], inputs["item_vecs"],
        inputs["W"], inputs["review_user_adj"], inputs["review_item_adj"],
        inputs["perm_u"], inputs["perm_i"],
        n_cores=N_CORES, rpc=RPC)



# revision 2
# speedup vs baseline: 1.8597x; 1.8597x over previous
"""Trainium2 Bass kernel for nn_ConcatenationAggregator.

For each review r:
    out[r] = relu(concat(review_vecs[r],
                         user_vecs[adj_u[r]][perm_u],
                         item_vecs[adj_i[r]][perm_i]) @ W)

Strategy (pure data-parallel over reviews, 8 NeuronCores):
  - The feature permutations AND the W2/W3 blocks of W are folded into the
    tables on the host: UP = user_vecs @ W2p, IP = item_vecs @ W3p.  Then
    out[r] = relu(review_vecs[r] @ W1 + UP[adj_u[r]] + IP[adj_i[r]]), i.e.
    the gathered rows are pure elementwise addends and the only device
    matmul left is the review term.
  - Everything on device is bf16 except the f32 PSUM accumulator: review
    stream, projected tables (rows padded to 128 elems = 256B so dma_gather
    accepts them), and the stored output.  This halves HBM/DMA traffic for
    the streamed tensors.
  - Row gathers use the GPSIMD `dma_gather` ucode (int16 indices, <=1024
    indices per call).  Since the tables exceed 32768 rows, the host sorts
    each core's reviews into 8 groups by (user-table 32K chunk, item-table
    32K chunk) so that rebased indices fit int16; the host un-permutes the
    output (identical scheme to the index layout dma_gather wants).
  - Layout: reviews are processed in 1024-row chunks of 8 sub-tiles.  The
    review stream is host-transposed to feature-major [64, slots]; each
    sub-tile j is a [64, 128] stationary lhsT and W1 the [64, 64] moving
    rhs, producing a row-major [128 rows, 64] PSUM block per sub-tile
    (one PSUM bank per chunk).  Gathered rows arrive row-major from
    dma_gather, so the user+item contributions are two DVE adds and the
    relu runs on the otherwise-idle Activation engine; output is stored
    row-major-wrapped [128, slots*64/128] and unwrapped on the host.
  - This toolchain build enforces ONE sync-wait slot per instruction, so
    tiny "observer" ops absorb extra cross-engine waits: a 1-column PE
    matmul takes the PSUM-recycle wait, small DVE/Act copies take the
    gather-completion and buffer-recycle waits, and the kernel-tail drain
    is split into single-wait drains.
"""

import os
import types

import numpy as np
import ml_dtypes

import concourse.bacc as bacc
import concourse.bass as bass
import concourse.mybir as mybir
import concourse.tile as tile
from concourse.bass_utils import run_bass_kernel_spmd
from concourse.vector_clock import ScopedClock, VectorClock

F32 = mybir.dt.float32
BF16 = mybir.dt.bfloat16
I16 = mybir.dt.int16

NP_BF16 = ml_dtypes.bfloat16

N_CORES = 8
D = 64
DPAD = 128                 # padded table row (256B in bf16, dma_gather min)
SUB = 128                  # reviews per sub-tile
MAX_S = 8                  # sub-tiles per chunk (<=1024 gather indices)
TCH = 32768                # table chunk (int16 index range)

N_REVIEWS = 1_000_000
N_USERS = 100_000
N_ITEMS = 50_000
RPC = N_REVIEWS // N_CORES

BUFS = int(os.environ.get("KBUFS", "3"))
PREF = int(os.environ.get("KPREF", "2"))


def _split_drain_and_barrier(self, tick_clock, wait_clock):
    """Replacement for TileContext._drain_and_barrier: the stock tail drain
    waits on every live proc semaphore at once, which overflows this
    toolchain's one-sync-wait-per-instruction limit.  Emit one drain per
    semaphore instead."""
    gc = tick_clock.global_clock
    ticks = list(gc)
    idxs = [i for i, t in enumerate(ticks) if t > 0]
    for i in idxs:
        sub = [0] * len(ticks)
        sub[i] = ticks[i]
        drain_inst = self.nc.sync.drain()
        wait_clock.add_sem_waits(
            drain_inst.ins, ScopedClock({None: VectorClock(sub)}))
    if not idxs:
        drain_inst = self.nc.sync.drain()
        wait_clock.add_sem_waits(
            drain_inst.ins, ScopedClock({None: VectorClock(ticks)}))
    self.nc.all_engine_barrier()
    assert self.sems is not None
    popped = self.nc._tile_sem_poison_stack.pop()
    assert popped is self._sem_poison
    self.nc.clear_and_free_semaphores(list(self.sems.allocated().values()))
    self.nc.all_engine_barrier()


def _chunk_list(s_per_group):
    """[(group, s_subtiles, row_base_slots, idxcol_base), ...] — shared by
    host packing and device program.  s values are even, <= MAX_S."""
    chunks = []
    row = 0
    col = 0
    for g, sg in enumerate(s_per_group):
        left = sg
        while left > 0:
            s = min(MAX_S, left)
            chunks.append((g, s, row, col))
            row += s * SUB
            col += s * 8
            left -= s
    return chunks


def _build_program(chunks, n_users, n_items):
    nc = bacc.Bacc("TRN2", target_bir_lowering=False, debug=False,
                   enable_asserts=False)
    padtot = sum(s for (_, s, _, _) in chunks) * SUB
    icols = padtot // 16

    rt_d = nc.dram_tensor("rt", [64, padtot], BF16, kind="ExternalInput")
    uidx_d = nc.dram_tensor("uidx", [128, icols], I16, kind="ExternalInput")
    iidx_d = nc.dram_tensor("iidx", [128, icols], I16, kind="ExternalInput")
    up_d = nc.dram_tensor("up", [n_users, DPAD], BF16, kind="ExternalInput")
    ip_d = nc.dram_tensor("ip", [n_items, DPAD], BF16, kind="ExternalInput")
    w1_d = nc.dram_tensor("w1", [64, 64], BF16, kind="ExternalInput")
    out_d = nc.dram_tensor("out2", [128, padtot // 2], BF16,
                           kind="ExternalOutput")

    T = len(chunks)
    RELU = mybir.ActivationFunctionType.Relu
    BYP = mybir.AluOpType.bypass
    ADD = mybir.AluOpType.add

    with tile.TileContext(nc) as tc:
        tc._drain_and_barrier = types.MethodType(_split_drain_and_barrier, tc)
        with tc.tile_pool(name="const", bufs=1) as constp, \
             tc.tile_pool(name="rtp", bufs=BUFS) as rtp, \
             tc.tile_pool(name="gup", bufs=BUFS) as gup, \
             tc.tile_pool(name="gip", bufs=BUFS) as gip, \
             tc.tile_pool(name="ttp", bufs=BUFS) as ttp, \
             tc.tile_pool(name="outp", bufs=BUFS) as outp, \
             tc.tile_pool(name="scr", bufs=1, space="PSUM") as scrp, \
             tc.tile_pool(name="mmp", bufs=BUFS, space="PSUM") as mmp:

            w1_t = constp.tile([64, 64], BF16)
            nc.sync.dma_start(out=w1_t[:], in_=w1_d.ap()[:, :])
            uidx_t = constp.tile([128, icols], I16)
            nc.sync.dma_start(out=uidx_t[:], in_=uidx_d.ap()[:, :])
            iidx_t = constp.tile([128, icols], I16)
            nc.sync.dma_start(out=iidx_t[:], in_=iidx_d.ap()[:, :])
            pscr = constp.tile([16, 16], I16)
            dscr = constp.tile([16, 16], BF16)
            ascr = constp.tile([16, 16], BF16)

            scratch = scrp.tile([128, 512], F32)
            # PE warmup: observe the weight-load (HWDGE) semaphore.
            nc.tensor.matmul(out=scratch[0:64, 0:64], lhsT=w1_t[:],
                             rhs=w1_t[:], start=True, stop=True)
            # Pool warmups: observe the index-table loads.
            nc.gpsimd.tensor_copy(out=pscr[:, :], in_=uidx_t[0:16, 0:16])
            nc.gpsimd.tensor_copy(out=pscr[:, :], in_=iidx_t[0:16, 0:16])

            nreg = {}
            for (_, s, _, _) in chunks:
                if s not in nreg:
                    nreg[s] = nc.gpsimd.to_reg(s * SUB)

            ubase = [g // 2 * TCH for g in range(8)]
            usize = [min(TCH, n_users - b) for b in ubase]
            ibase = [g % 2 * TCH for g in range(8)]
            isize = [min(TCH, n_items - b) for b in ibase]

            rt_tiles = [None] * T
            gu_tiles = [None] * T
            gi_tiles = [None] * T
            ps_tiles = [None] * T
            o_tiles = [None] * T

            def issue_loads(t):
                g, s, row, col = chunks[t]
                rt_t = rtp.tile([64, MAX_S * 128], BF16, tag="rt")
                nc.sync.dma_start(
                    out=rt_t[:, :s * 128],
                    in_=rt_d.ap()[:, row: row + s * 128])
                gu_t = gup.tile([128, MAX_S * DPAD], BF16, tag="gu")
                nc.gpsimd.dma_gather(
                    out_ap=gu_t[:, :s * DPAD].rearrange(
                        "p (n d) -> p n d", d=DPAD),
                    in_ap=up_d.ap()[ubase[g]:ubase[g] + usize[g], :],
                    idxs_ap=uidx_t[:, col:col + s * 8],
                    num_idxs=s * SUB, num_idxs_reg=nreg[s], elem_size=DPAD)
                gi_t = gip.tile([128, MAX_S * DPAD], BF16, tag="gi")
                nc.gpsimd.dma_gather(
                    out_ap=gi_t[:, :s * DPAD].rearrange(
                        "p (n d) -> p n d", d=DPAD),
                    in_ap=ip_d.ap()[ibase[g]:ibase[g] + isize[g], :],
                    idxs_ap=iidx_t[:, col:col + s * 8],
                    num_idxs=s * SUB, num_idxs_reg=nreg[s], elem_size=DPAD)
                rt_tiles[t], gu_tiles[t], gi_tiles[t] = rt_t, gu_t, gi_t

            def issue_matmuls(t):
                _, s, _, _ = chunks[t]
                rt_t = rt_tiles[t]
                ps = mmp.tile([128, MAX_S * 64], F32, tag="mm")
                # 1-column dummy absorbs the PSUM-recycle wait so the first
                # real matmul carries only the rt-load wait.
                nc.tensor.matmul(out=ps[0:64, 0:1], lhsT=w1_t[:],
                                 rhs=w1_t[:, 0:1], start=True, stop=True)
                for j in range(s):
                    nc.tensor.matmul(
                        out=ps[:, j * 64:(j + 1) * 64],
                        lhsT=rt_t[:, j * 128:(j + 1) * 128],
                        rhs=w1_t[:], start=True, stop=True)
                ps_tiles[t] = ps

            def issue_elemwise(t):
                _, s, _, _ = chunks[t]
                n = s * 64
                gu_t, gi_t = gu_tiles[t], gi_tiles[t]
                ps = ps_tiles[t]
                # Observers: absorb the two gather-completion waits one at a
                # time so each real DVE op keeps a single wait slot.
                nc.vector.tensor_copy(out=dscr[:, :], in_=gu_t[0:16, 0:16])
                t_t = ttp.tile([128, MAX_S * 64], BF16, tag="t")
                gu3 = gu_t[:, :s * DPAD].rearrange("p (n d) -> p n d", d=DPAD)
                gi3 = gi_t[:, :s * DPAD].rearrange("p (n d) -> p n d", d=DPAD)
                t3 = t_t[:, :n].rearrange("p (n d) -> p n d", d=64)
                nc.vector.scalar_tensor_tensor(
                    out=t3, in0=gu3[:, :, 0:64], scalar=0.0,
                    in1=gi3[:, :, 0:64], op0=BYP, op1=ADD)
                # psum += t  (in-place on PSUM; waits only on the last matmul)
                nc.vector.scalar_tensor_tensor(
                    out=ps[:, :n], in0=ps[:, :n], scalar=0.0,
                    in1=t_t[:, :n], op0=BYP, op1=ADD)

            def issue_relu(t):
                _, s, row, _ = chunks[t]
                n = s * 64
                o_t = outp.tile([128, MAX_S * 64], BF16, tag="o")
                # Observer: absorb the o_t store-recycle wait.
                nc.scalar.activation(out=ascr[:, :], in_=ascr[:, :],
                                     func=RELU)
                nc.scalar.activation(out=o_t[:, :n], in_=ps_tiles[t][:, :n],
                                     func=RELU)
                o_tiles[t] = o_t

            def issue_store(t):
                _, s, row, _ = chunks[t]
                nc.sync.dma_start(
                    out=out_d.ap()[:, row // 2: row // 2 + s * 64],
                    in_=o_tiles[t][:, :s * 64])

            # Software-pipelined emission (see module docstring).
            for tt in range(min(PREF, T)):
                issue_loads(tt)
            for t in range(T):
                if t + PREF < T:
                    issue_loads(t + PREF)
                issue_matmuls(t)
                issue_elemwise(t)
                issue_relu(t)
                issue_store(t)
    nc.finalize()
    return nc


_PROGRAM_CACHE: dict = {}


def _get_program(chunk_key, n_users, n_items):
    key = (chunk_key, n_users, n_items)
    if key not in _PROGRAM_CACHE:
        _PROGRAM_CACHE[key] = (
            _build_program(_chunk_list(list(chunk_key)), n_users, n_items))
    return _PROGRAM_CACHE[key]


def _wrap_idx(flat_sorted, chunks):
    """Rebased int16 indices [PADTOT] -> [128, PADTOT//16] in dma_gather's
    wrapped layout: per chunk block [128, 8*s] with block[p, m] =
    flat[m*16 + p%16], replicated across the 8 16-partition groups."""
    cols = []
    for (_, s, row, _) in chunks:
        blk = flat_sorted[row:row + s * SUB].reshape(s * 8, 16).T  # [16, 8s]
        cols.append(np.tile(blk, (8, 1)))
    return np.ascontiguousarray(np.concatenate(cols, axis=1))


def _run(review_vecs, user_vecs, item_vecs, W,
         review_user_adj, review_item_adj, perm_u, perm_i,
         n_cores, rpc):
    n_users = user_vecs.shape[0]
    n_items = item_vecs.shape[0]

    W = np.asarray(W, np.float32)
    W1 = np.ascontiguousarray(W[0:64])
    W2 = W[64:128]
    W3 = W[128:192]
    perm_u = np.asarray(perm_u, np.int64)
    perm_i = np.asarray(perm_i, np.int64)
    W2p = np.empty_like(W2)
    W2p[perm_u] = W2
    W3p = np.empty_like(W3)
    W3p[perm_i] = W3

    user_vecs = np.asarray(user_vecs, np.float32)
    item_vecs = np.asarray(item_vecs, np.float32)
    # Fold W2p/W3p into the tables; pad rows to 128 elems (256B in bf16).
    UP = np.zeros((n_users, DPAD), NP_BF16)
    UP[:, :64] = (user_vecs @ W2p).astype(NP_BF16)
    IP = np.zeros((n_items, DPAD), NP_BF16)
    IP[:, :64] = (item_vecs @ W3p).astype(NP_BF16)
    W1b = np.ascontiguousarray(W1.astype(NP_BF16))

    review_vecs = np.asarray(review_vecs, np.float32)
    au_all = np.asarray(review_user_adj, np.int64)
    ai_all = np.asarray(review_item_adj, np.int64)

    # Group each core's reviews by (user 32K chunk, item 32K chunk).
    per_core = []
    s_max = np.zeros(8, np.int64)
    for c in range(n_cores):
        lo = c * rpc
        au = au_all[lo:lo + rpc]
        ai = ai_all[lo:lo + rpc]
        grp = (au // TCH) * 2 + (ai // TCH)
        order = np.argsort(grp, kind="stable")
        counts = np.bincount(grp, minlength=8)
        per_core.append((order, counts))
        s_max = np.maximum(s_max, -(-counts // SUB))
    # shared chunk structure: even sub-tile counts per group
    s_per_group = [int(s + (s % 2)) for s in s_max]
    chunk_key = tuple(s_per_group)
    chunks = _chunk_list(s_per_group)
    padtot = sum(s for (_, s, _, _) in chunks) * SUB

    nc = _get_program(chunk_key, n_users, n_items)

    in_maps = []
    slotmaps = []
    for c in range(n_cores):
        lo = c * rpc
        au = au_all[lo:lo + rpc]
        ai = ai_all[lo:lo + rpc]
        order, counts = per_core[c]
        slotmap = np.full(padtot, -1, np.int64)
        row = 0
        start = 0
        for g in range(8):
            cnt = int(counts[g])
            ids = order[start:start + cnt]
            slotmap[row:row + cnt] = ids
            start += cnt
            row += s_per_group[g] * SUB
        valid = slotmap >= 0
        sl = np.where(valid, slotmap, 0)

        rv_sorted = np.where(valid[:, None],
                             review_vecs[lo:lo + rpc][sl], 0.0)
        slot_g = np.repeat(np.arange(8), np.array(s_per_group) * SUB)
        u_reb = np.where(valid, au[sl] - (slot_g // 2) * TCH, 0).astype(np.int16)
        i_reb = np.where(valid, ai[sl] - (slot_g % 2) * TCH, 0).astype(np.int16)

        in_maps.append({
            "rt": np.ascontiguousarray(rv_sorted.T.astype(NP_BF16)),
            "uidx": _wrap_idx(u_reb, chunks),
            "iidx": _wrap_idx(i_reb, chunks),
            "up": UP,
            "ip": IP,
            "w1": W1b,
        })
        slotmaps.append((slotmap, valid))

    res = run_bass_kernel_spmd(nc, in_maps, core_ids=list(range(n_cores)))

    out = np.empty((n_cores * rpc, 64), np.float32)
    for c in range(n_cores):
        o2 = np.asarray(res.results[c]["out2"])
        # [128, padtot//2] wrapped row-major -> [padtot, 64]
        out_sorted = np.ascontiguousarray(
            o2.reshape(128, padtot // 128, 64).transpose(1, 0, 2)
        ).reshape(padtot, 64).astype(np.float32)
        slotmap, valid = slotmaps[c]
        out[c * rpc + slotmap[valid]] = out_sorted[valid]
    return out


def kernel(**inputs) -> np.ndarray:
    return _run(
        inputs["review_vecs"], inputs["user_vecs"], inputs["item_vecs"],
        inputs["W"], inputs["review_user_adj"], inputs["review_item_adj"],
        inputs["perm_u"], inputs["perm_i"],
        n_cores=N_CORES, rpc=RPC)


# revision 14
# speedup vs baseline: 1.8876x; 1.0150x over previous
"""Trainium2 Bass kernel for nn_ConcatenationAggregator.

For each review r:
    out[r] = relu(concat(review_vecs[r],
                         user_vecs[adj_u[r]][perm_u],
                         item_vecs[adj_i[r]][perm_i]) @ W)

Strategy (pure data-parallel over reviews, 8 NeuronCores):
  - The feature permutations AND the W2/W3 blocks of W are folded into the
    tables on the host: UP = user_vecs @ W2p, IP = item_vecs @ W3p.  Then
    out[r] = relu(review_vecs[r] @ W1 + UP[adj_u[r]] + IP[adj_i[r]]), i.e.
    the gathered rows are pure elementwise addends and the only device
    matmul left is the review term.
  - Everything on device is bf16 except the f32 PSUM accumulator: review
    stream, projected tables (rows padded to 128 elems = 256B so dma_gather
    accepts them), and the stored output.  This halves HBM/DMA traffic for
    the streamed tensors.
  - Row gathers use the GPSIMD `dma_gather` ucode (int16 indices, <=1024
    indices per call).  Since the tables exceed 32768 rows, the host sorts
    each core's reviews into 8 groups by (user-table 32K chunk, item-table
    32K chunk) so that rebased indices fit int16; the host un-permutes the
    output (identical scheme to the index layout dma_gather wants).
  - Layout: reviews are processed in 1024-row chunks of 8 sub-tiles.  The
    review stream is host-transposed to feature-major [64, slots]; each
    sub-tile j is a [64, 128] stationary lhsT and W1 the [64, 64] moving
    rhs, producing a row-major [128 rows, 64] PSUM block per sub-tile
    (one PSUM bank per chunk).  Gathered rows arrive row-major from
    dma_gather, so the user+item contributions are two DVE adds and the
    relu runs on the otherwise-idle Activation engine; output is stored
    row-major-wrapped [128, slots*64/128] and unwrapped on the host.
  - This toolchain build enforces ONE sync-wait slot per instruction, so
    tiny "observer" ops absorb extra cross-engine waits: a 1-column PE
    matmul takes the PSUM-recycle wait, small DVE/Act copies take the
    gather-completion and buffer-recycle waits, and the kernel-tail drain
    is split into single-wait drains.
"""

import os
import types

import numpy as np
import ml_dtypes

import concourse.bacc as bacc
import concourse.bass as bass
import concourse.mybir as mybir
import concourse.tile as tile
from concourse.bass_utils import run_bass_kernel_spmd
from concourse.vector_clock import ScopedClock, VectorClock

F32 = mybir.dt.float32
BF16 = mybir.dt.bfloat16
I16 = mybir.dt.int16

NP_BF16 = ml_dtypes.bfloat16

N_CORES = 8
D = 64
DPAD = 128                 # padded table row (256B in bf16, dma_gather min)
SUB = 128                  # reviews per sub-tile
MAX_S = 8                  # sub-tiles per chunk (<=1024 gather indices)
TCH = 32768                # table chunk (int16 index range)

N_REVIEWS = 1_000_000
N_USERS = 100_000
N_ITEMS = 50_000
RPC = N_REVIEWS // N_CORES

BUFS = int(os.environ.get("KBUFS", "5"))
PREF = int(os.environ.get("KPREF", "4"))


def _split_drain_and_barrier(self, tick_clock, wait_clock):
    """Replacement for TileContext._drain_and_barrier: the stock tail drain
    waits on every live proc semaphore at once, which overflows this
    toolchain's one-sync-wait-per-instruction limit.  Emit one drain per
    semaphore instead."""
    gc = tick_clock.global_clock
    ticks = list(gc)
    idxs = [i for i, t in enumerate(ticks) if t > 0]
    for i in idxs:
        sub = [0] * len(ticks)
        sub[i] = ticks[i]
        drain_inst = self.nc.sync.drain()
        wait_clock.add_sem_waits(
            drain_inst.ins, ScopedClock({None: VectorClock(sub)}))
    if not idxs:
        drain_inst = self.nc.sync.drain()
        wait_clock.add_sem_waits(
            drain_inst.ins, ScopedClock({None: VectorClock(ticks)}))
    self.nc.all_engine_barrier()
    assert self.sems is not None
    popped = self.nc._tile_sem_poison_stack.pop()
    assert popped is self._sem_poison
    self.nc.clear_and_free_semaphores(list(self.sems.allocated().values()))
    self.nc.all_engine_barrier()


def _chunk_list(s_per_group):
    """[(group, s_subtiles, row_base_slots, idxcol_base), ...] — shared by
    host packing and device program.  s values are <= MAX_S."""
    chunks = []
    row = 0
    col = 0
    for g, sg in enumerate(s_per_group):
        left = sg
        while left > 0:
            s = min(MAX_S, left)
            chunks.append((g, s, row, col))
            row += s * SUB
            col += s * 8
            left -= s
    return chunks


def _build_program(chunks, n_users, n_items):
    nc = bacc.Bacc("TRN2", target_bir_lowering=False, debug=False,
                   enable_asserts=False)
    padtot = sum(s for (_, s, _, _) in chunks) * SUB
    icols = padtot // 16

    rt_d = nc.dram_tensor("rt", [64, padtot], BF16, kind="ExternalInput")
    uidx_d = nc.dram_tensor("uidx", [16, icols], I16, kind="ExternalInput")
    iidx_d = nc.dram_tensor("iidx", [16, icols], I16, kind="ExternalInput")
    up_d = nc.dram_tensor("up", [n_users, DPAD], BF16, kind="ExternalInput")
    ip_d = nc.dram_tensor("ip", [n_items, DPAD], BF16, kind="ExternalInput")
    w1_d = nc.dram_tensor("w1", [64, 64], BF16, kind="ExternalInput")
    out_d = nc.dram_tensor("out2", [128, padtot // 2], BF16,
                           kind="ExternalOutput")

    T = len(chunks)
    RELU = mybir.ActivationFunctionType.Relu
    BYP = mybir.AluOpType.bypass
    ADD = mybir.AluOpType.add

    with tile.TileContext(nc) as tc:
        tc._drain_and_barrier = types.MethodType(_split_drain_and_barrier, tc)
        with tc.tile_pool(name="const", bufs=1) as constp, \
             tc.tile_pool(name="rtp", bufs=BUFS) as rtp, \
             tc.tile_pool(name="gup", bufs=BUFS) as gup, \
             tc.tile_pool(name="gip", bufs=BUFS) as gip, \
             tc.tile_pool(name="ttp", bufs=BUFS) as ttp, \
             tc.tile_pool(name="outp", bufs=BUFS) as outp, \
             tc.tile_pool(name="scr", bufs=1, space="PSUM") as scrp, \
             tc.tile_pool(name="mmp", bufs=BUFS, space="PSUM") as mmp:

            w1_t = constp.tile([64, 64], BF16)
            nc.sync.dma_start(out=w1_t[:], in_=w1_d.ap()[:, :])
            # Index tables: the gather ucode wants the 16-partition-wrapped
            # indices replicated across all 8 GPSIMD cores.  Only the unique
            # 16 partitions travel over DMA (per-group slices, so the first
            # gathers don't stall on the full upload); the 8x replication
            # runs on the lightly-loaded DVE engine.  Engine APs may only
            # start at partition 0/32/64/96, so DMA fills [0:16] and [16:32]
            # and DVE doubles [0:32]->[32:64] and [0:64]->[64:128].
            uidx_t = constp.tile([128, icols], I16)
            iidx_t = constp.tile([128, icols], I16)
            gcols: dict = {}
            for (g, s, _, col) in chunks:
                c0, c1 = gcols.get(g, (col, col))
                gcols[g] = (min(c0, col), max(c1, col + s * 8))
            pscr = constp.tile([16, 16], I16)
            dscr = constp.tile([16, 16], BF16)
            ascr = constp.tile([16, 16], BF16)

            def fill_idx(g):
                c0, c1 = gcols[g]
                for src_d, dst_t in ((uidx_d, uidx_t), (iidx_d, iidx_t)):
                    nc.sync.dma_start(out=dst_t[0:16, c0:c1],
                                      in_=src_d.ap()[:, c0:c1])
                    nc.scalar.dma_start(out=dst_t[16:32, c0:c1],
                                        in_=src_d.ap()[:, c0:c1])
                    # One HWDGE wait (max tick of the two loads) lands on the
                    # first copy; the second and the gathers ride the DVE
                    # clock.
                    nc.vector.tensor_copy(out=dst_t[32:64, c0:c1],
                                          in_=dst_t[0:32, c0:c1])
                    nc.vector.tensor_copy(out=dst_t[64:128, c0:c1],
                                          in_=dst_t[0:64, c0:c1])

            scratch = scrp.tile([128, 512], F32)
            # PE warmup: observe the weight-load (HWDGE) semaphore.
            nc.tensor.matmul(out=scratch[0:64, 0:64], lhsT=w1_t[:],
                             rhs=w1_t[:], start=True, stop=True)

            nreg = {}
            for (_, s, _, _) in chunks:
                if s not in nreg:
                    nreg[s] = nc.gpsimd.to_reg(s * SUB)

            ubase = [g // 2 * TCH for g in range(8)]
            usize = [min(TCH, n_users - b) for b in ubase]
            ibase = [g % 2 * TCH for g in range(8)]
            isize = [min(TCH, n_items - b) for b in ibase]

            rt_tiles = [None] * T
            gu_tiles = [None] * T
            gi_tiles = [None] * T
            ps_tiles = [None] * T
            o_tiles = [None] * T

            last_g = [-1]

            def issue_loads(t):
                g, s, row, col = chunks[t]
                if g != last_g[0]:
                    fill_idx(g)
                    last_g[0] = g
                rt_t = rtp.tile([64, MAX_S * 128], BF16, tag="rt")
                nc.sync.dma_start(
                    out=rt_t[:, :s * 128],
                    in_=rt_d.ap()[:, row: row + s * 128])
                gu_t = gup.tile([128, MAX_S * DPAD], BF16, tag="gu")
                nc.gpsimd.dma_gather(
                    out_ap=gu_t[:, :s * DPAD].rearrange(
                        "p (n d) -> p n d", d=DPAD),
                    in_ap=up_d.ap()[ubase[g]:ubase[g] + usize[g], :],
                    idxs_ap=uidx_t[:, col:col + s * 8],
                    num_idxs=s * SUB, num_idxs_reg=nreg[s], elem_size=DPAD)
                gi_t = gip.tile([128, MAX_S * DPAD], BF16, tag="gi")
                nc.gpsimd.dma_gather(
                    out_ap=gi_t[:, :s * DPAD].rearrange(
                        "p (n d) -> p n d", d=DPAD),
                    in_ap=ip_d.ap()[ibase[g]:ibase[g] + isize[g], :],
                    idxs_ap=iidx_t[:, col:col + s * 8],
                    num_idxs=s * SUB, num_idxs_reg=nreg[s], elem_size=DPAD)
                rt_tiles[t], gu_tiles[t], gi_tiles[t] = rt_t, gu_t, gi_t

            def issue_matmuls(t):
                _, s, _, _ = chunks[t]
                rt_t = rt_tiles[t]
                ps = mmp.tile([128, MAX_S * 64], F32, tag="mm")
                # 1-column dummy absorbs the PSUM-recycle wait so the first
                # real matmul carries only the rt-load wait.
                nc.tensor.matmul(out=ps[0:64, 0:1], lhsT=w1_t[:],
                                 rhs=w1_t[:, 0:1], start=True, stop=True)
                for j in range(s):
                    nc.tensor.matmul(
                        out=ps[:, j * 64:(j + 1) * 64],
                        lhsT=rt_t[:, j * 128:(j + 1) * 128],
                        rhs=w1_t[:], start=True, stop=True)
                ps_tiles[t] = ps

            def issue_elemwise(t):
                _, s, _, _ = chunks[t]
                n = s * 64
                gu_t, gi_t = gu_tiles[t], gi_tiles[t]
                ps = ps_tiles[t]
                # Observers: absorb the two gather-completion waits one at a
                # time so each real DVE op keeps a single wait slot.
                nc.vector.tensor_copy(out=dscr[:, :], in_=gu_t[0:16, 0:16])
                t_t = ttp.tile([128, MAX_S * 64], BF16, tag="t")
                gu3 = gu_t[:, :s * DPAD].rearrange("p (n d) -> p n d", d=DPAD)
                gi3 = gi_t[:, :s * DPAD].rearrange("p (n d) -> p n d", d=DPAD)
                t3 = t_t[:, :n].rearrange("p (n d) -> p n d", d=64)
                nc.vector.scalar_tensor_tensor(
                    out=t3, in0=gu3[:, :, 0:64], scalar=0.0,
                    in1=gi3[:, :, 0:64], op0=BYP, op1=ADD)
                # psum += t  (in-place on PSUM; waits only on the last matmul)
                nc.vector.scalar_tensor_tensor(
                    out=ps[:, :n], in0=ps[:, :n], scalar=0.0,
                    in1=t_t[:, :n], op0=BYP, op1=ADD)

            def issue_relu(t):
                _, s, row, _ = chunks[t]
                n = s * 64
                o_t = outp.tile([128, MAX_S * 64], BF16, tag="o")
                # Observer: absorb the o_t store-recycle wait.
                nc.scalar.activation(out=ascr[:, :], in_=ascr[:, :],
                                     func=RELU)
                nc.scalar.activation(out=o_t[:, :n], in_=ps_tiles[t][:, :n],
                                     func=RELU)
                o_tiles[t] = o_t

            def issue_store(t):
                # Issued from the Activation HWDGE queue: the wait on the
                # relu is same-engine there, so SP's sequencer never blocks
                # ahead of the next chunk's loads.
                _, s, row, _ = chunks[t]
                nc.scalar.dma_start(
                    out=out_d.ap()[:, row // 2: row // 2 + s * 64],
                    in_=o_tiles[t][:, :s * 64])

            # Software-pipelined emission (see module docstring).
            for tt in range(min(PREF, T)):
                issue_loads(tt)
            for t in range(T):
                if t + PREF < T:
                    issue_loads(t + PREF)
                issue_matmuls(t)
                issue_elemwise(t)
                issue_relu(t)
                issue_store(t)
    nc.finalize()
    return nc


_PROGRAM_CACHE: dict = {}


def _get_program(chunk_key, n_users, n_items):
    key = (chunk_key, n_users, n_items)
    if key not in _PROGRAM_CACHE:
        _PROGRAM_CACHE[key] = (
            _build_program(_chunk_list(list(chunk_key)), n_users, n_items))
    return _PROGRAM_CACHE[key]


def _wrap_idx(flat_sorted, chunks):
    """Rebased int16 indices [PADTOT] -> [16, PADTOT//16] in dma_gather's
    wrapped layout: per chunk block [16, 8*s] with block[p, m] =
    flat[m*16 + p]; the device replicates across the 8 16-partition
    groups."""
    cols = []
    for (_, s, row, _) in chunks:
        blk = flat_sorted[row:row + s * SUB].reshape(s * 8, 16).T  # [16, 8s]
        cols.append(blk)
    return np.ascontiguousarray(np.concatenate(cols, axis=1))


def _run(review_vecs, user_vecs, item_vecs, W,
         review_user_adj, review_item_adj, perm_u, perm_i,
         n_cores, rpc):
    n_users = user_vecs.shape[0]
    n_items = item_vecs.shape[0]

    W = np.asarray(W, np.float32)
    W1 = np.ascontiguousarray(W[0:64])
    W2 = W[64:128]
    W3 = W[128:192]
    perm_u = np.asarray(perm_u, np.int64)
    perm_i = np.asarray(perm_i, np.int64)
    W2p = np.empty_like(W2)
    W2p[perm_u] = W2
    W3p = np.empty_like(W3)
    W3p[perm_i] = W3

    user_vecs = np.asarray(user_vecs, np.float32)
    item_vecs = np.asarray(item_vecs, np.float32)
    # Fold W2p/W3p into the tables; pad rows to 128 elems (256B in bf16).
    UP = np.zeros((n_users, DPAD), NP_BF16)
    UP[:, :64] = (user_vecs @ W2p).astype(NP_BF16)
    IP = np.zeros((n_items, DPAD), NP_BF16)
    IP[:, :64] = (item_vecs @ W3p).astype(NP_BF16)
    W1b = np.ascontiguousarray(W1.astype(NP_BF16))

    review_vecs = np.asarray(review_vecs, np.float32)
    au_all = np.asarray(review_user_adj, np.int64)
    ai_all = np.asarray(review_item_adj, np.int64)

    # Group each core's reviews by (user 32K chunk, item 32K chunk).
    per_core = []
    s_max = np.zeros(8, np.int64)
    for c in range(n_cores):
        lo = c * rpc
        au = au_all[lo:lo + rpc]
        ai = ai_all[lo:lo + rpc]
        grp = (au // TCH) * 2 + (ai // TCH)
        order = np.argsort(grp, kind="stable")
        counts = np.bincount(grp, minlength=8)
        per_core.append((order, counts))
        s_max = np.maximum(s_max, -(-counts // SUB))
    # shared chunk structure: per-group sub-tile counts (max over cores)
    s_per_group = [int(s) for s in s_max]
    chunk_key = tuple(s_per_group)
    chunks = _chunk_list(s_per_group)
    padtot = sum(s for (_, s, _, _) in chunks) * SUB

    nc = _get_program(chunk_key, n_users, n_items)

    in_maps = []
    slotmaps = []
    for c in range(n_cores):
        lo = c * rpc
        au = au_all[lo:lo + rpc]
        ai = ai_all[lo:lo + rpc]
        order, counts = per_core[c]
        slotmap = np.full(padtot, -1, np.int64)
        row = 0
        start = 0
        for g in range(8):
            cnt = int(counts[g])
            ids = order[start:start + cnt]
            slotmap[row:row + cnt] = ids
            start += cnt
            row += s_per_group[g] * SUB
        valid = slotmap >= 0
        sl = np.where(valid, slotmap, 0)

        rv_sorted = np.where(valid[:, None],
                             review_vecs[lo:lo + rpc][sl], 0.0)
        slot_g = np.repeat(np.arange(8), np.array(s_per_group) * SUB)
        u_reb = np.where(valid, au[sl] - (slot_g // 2) * TCH, 0).astype(np.int16)
        i_reb = np.where(valid, ai[sl] - (slot_g % 2) * TCH, 0).astype(np.int16)

        in_maps.append({
            "rt": np.ascontiguousarray(rv_sorted.T.astype(NP_BF16)),
            "uidx": _wrap_idx(u_reb, chunks),
            "iidx": _wrap_idx(i_reb, chunks),
            "up": UP,
            "ip": IP,
            "w1": W1b,
        })
        slotmaps.append((slotmap, valid))

    res = run_bass_kernel_spmd(nc, in_maps, core_ids=list(range(n_cores)))

    out = np.empty((n_cores * rpc, 64), np.float32)
    for c in range(n_cores):
        o2 = np.asarray(res.results[c]["out2"])
        # [128, padtot//2] wrapped row-major -> [padtot, 64]
        out_sorted = np.ascontiguousarray(
            o2.reshape(128, padtot // 128, 64).transpose(1, 0, 2)
        ).reshape(padtot, 64).astype(np.float32)
        slotmap, valid = slotmaps[c]
        out[c * rpc + slotmap[valid]] = out_sorted[valid]
    return out


def kernel(**inputs) -> np.ndarray:
    return _run(
        inputs["review_vecs"], inputs["user_vecs"], inputs["item_vecs"],
        inputs["W"], inputs["review_user_adj"], inputs["review_item_adj"],
        inputs["perm_u"], inputs["perm_i"],
        n_cores=N_CORES, rpc=RPC)


# revision 25
# speedup vs baseline: 1.9071x; 1.0104x over previous
"""Trainium2 Bass kernel for nn_ConcatenationAggregator.

For each review r:
    out[r] = relu(concat(review_vecs[r],
                         user_vecs[adj_u[r]][perm_u],
                         item_vecs[adj_i[r]][perm_i]) @ W)

Strategy (pure data-parallel over reviews, 8 NeuronCores):
  - The feature permutations AND the W2/W3 blocks of W are folded into the
    tables on the host: UP = user_vecs @ W2p, IP = item_vecs @ W3p.  Then
    out[r] = relu(review_vecs[r] @ W1 + UP[adj_u[r]] + IP[adj_i[r]]), i.e.
    the gathered rows are pure elementwise addends and the only device
    matmul left is the review term.
  - Everything on device is bf16 except the f32 PSUM accumulator: review
    stream, projected tables (rows padded to 128 elems = 256B so dma_gather
    accepts them), and the stored output.  This halves HBM/DMA traffic for
    the streamed tensors.
  - Row gathers use the GPSIMD `dma_gather` ucode (int16 indices, <=1024
    indices per call).  Since the tables exceed 32768 rows, the host sorts
    each core's reviews into 8 groups by (user-table 32K chunk, item-table
    32K chunk) so that rebased indices fit int16; the host un-permutes the
    output (identical scheme to the index layout dma_gather wants).
  - Layout: reviews are processed in 1024-row chunks of 8 sub-tiles.  The
    review stream is host-transposed to feature-major [64, slots]; each
    sub-tile j is a [64, 128] stationary lhsT and W1 the [64, 64] moving
    rhs, producing a row-major [128 rows, 64] PSUM block per sub-tile
    (one PSUM bank per chunk).  Gathered rows arrive row-major from
    dma_gather, so the user+item contributions are two DVE adds and the
    relu runs on the otherwise-idle Activation engine; output is stored
    row-major-wrapped [128, slots*64/128] and unwrapped on the host.
  - This toolchain build enforces ONE sync-wait slot per instruction, so
    tiny "observer" ops absorb extra cross-engine waits: a 1-column PE
    matmul takes the PSUM-recycle wait, small DVE/Act copies take the
    gather-completion and buffer-recycle waits, and the kernel-tail drain
    is split into single-wait drains.
"""

import os
import types

import numpy as np
import ml_dtypes

import concourse.bacc as bacc
import concourse.bass as bass
import concourse.mybir as mybir
import concourse.tile as tile
from concourse.bass_utils import run_bass_kernel_spmd
from concourse.vector_clock import ScopedClock, VectorClock

F32 = mybir.dt.float32
BF16 = mybir.dt.bfloat16
I16 = mybir.dt.int16

NP_BF16 = ml_dtypes.bfloat16

N_CORES = 8
D = 64
DPAD = 128                 # padded table row (256B in bf16, dma_gather min)
SUB = 128                  # reviews per sub-tile
MAX_S = 8                  # sub-tiles per chunk (<=1024 gather indices)
TCH = 32768                # table chunk (int16 index range)

N_REVIEWS = 1_000_000
N_USERS = 100_000
N_ITEMS = 50_000
RPC = N_REVIEWS // N_CORES

BUFS = int(os.environ.get("KBUFS", "5"))
PREF = int(os.environ.get("KPREF", "4"))


def _split_drain_and_barrier(self, tick_clock, wait_clock):
    """Replacement for TileContext._drain_and_barrier: the stock tail drain
    waits on every live proc semaphore at once, which overflows this
    toolchain's one-sync-wait-per-instruction limit.  Emit one drain per
    semaphore instead."""
    gc = tick_clock.global_clock
    ticks = list(gc)
    idxs = [i for i, t in enumerate(ticks) if t > 0]
    for i in idxs:
        sub = [0] * len(ticks)
        sub[i] = ticks[i]
        drain_inst = self.nc.sync.drain()
        wait_clock.add_sem_waits(
            drain_inst.ins, ScopedClock({None: VectorClock(sub)}))
    if not idxs:
        drain_inst = self.nc.sync.drain()
        wait_clock.add_sem_waits(
            drain_inst.ins, ScopedClock({None: VectorClock(ticks)}))
    self.nc.all_engine_barrier()
    assert self.sems is not None
    popped = self.nc._tile_sem_poison_stack.pop()
    assert popped is self._sem_poison
    self.nc.clear_and_free_semaphores(list(self.sems.allocated().values()))
    self.nc.all_engine_barrier()


def _chunk_list(s_per_group):
    """[(group, s_subtiles, row_base_slots, idxcol_base), ...] — shared by
    host packing and device program.  s values are <= MAX_S."""
    chunks = []
    row = 0
    col = 0
    for g, sg in enumerate(s_per_group):
        left = sg
        while left > 0:
            s = min(MAX_S, left)
            chunks.append((g, s, row, col))
            row += s * SUB
            col += s * 8
            left -= s
    return chunks


def _build_program(chunks, n_users, n_items):
    nc = bacc.Bacc("TRN2", target_bir_lowering=False, debug=False,
                   enable_asserts=False)
    padtot = sum(s for (_, s, _, _) in chunks) * SUB
    icols = padtot // 16

    rt_d = nc.dram_tensor("rt", [64, padtot], BF16, kind="ExternalInput")
    uidx_d = nc.dram_tensor("uidx", [16, icols], I16, kind="ExternalInput")
    iidx_d = nc.dram_tensor("iidx", [16, icols], I16, kind="ExternalInput")
    up_d = nc.dram_tensor("up", [n_users, DPAD], BF16, kind="ExternalInput")
    ip_d = nc.dram_tensor("ip", [n_items, DPAD], BF16, kind="ExternalInput")
    w1_d = nc.dram_tensor("w1", [64, 64], BF16, kind="ExternalInput")
    out_d = nc.dram_tensor("out2", [128, padtot // 2], BF16,
                           kind="ExternalOutput")

    T = len(chunks)
    RELU = mybir.ActivationFunctionType.Relu
    BYP = mybir.AluOpType.bypass
    ADD = mybir.AluOpType.add

    with tile.TileContext(nc) as tc:
        tc._drain_and_barrier = types.MethodType(_split_drain_and_barrier, tc)
        with tc.tile_pool(name="const", bufs=1) as constp, \
             tc.tile_pool(name="rtp", bufs=BUFS) as rtp, \
             tc.tile_pool(name="gup", bufs=BUFS) as gup, \
             tc.tile_pool(name="gip", bufs=BUFS) as gip, \
             tc.tile_pool(name="ttp", bufs=BUFS) as ttp, \
             tc.tile_pool(name="outp", bufs=BUFS) as outp, \
             tc.tile_pool(name="scr", bufs=1, space="PSUM") as scrp, \
             tc.tile_pool(name="mmp", bufs=BUFS, space="PSUM") as mmp:

            w1_t = constp.tile([64, 64], BF16)
            nc.sync.dma_start(out=w1_t[:], in_=w1_d.ap()[:, :])
            # Index tables: the gather ucode wants the 16-partition-wrapped
            # indices replicated across all 8 GPSIMD cores.  Only the unique
            # 16 partitions travel over DMA (per-group slices, so the first
            # gathers don't stall on the full upload); the 8x replication
            # runs on the lightly-loaded DVE engine.  Engine APs may only
            # start at partition 0/32/64/96, so DMA fills [0:16] and [16:32]
            # and DVE doubles [0:32]->[32:64] and [0:64]->[64:128].
            uidx_t = constp.tile([128, icols], I16)
            iidx_t = constp.tile([128, icols], I16)
            gcols: dict = {}
            for (g, s, _, col) in chunks:
                c0, c1 = gcols.get(g, (col, col))
                gcols[g] = (min(c0, col), max(c1, col + s * 8))
            dscr = constp.tile([16, 16], BF16)
            ascr = constp.tile([16, 16], BF16)

            def fill_idx(g):
                c0, c1 = gcols[g]
                for src_d, dst_t in ((uidx_d, uidx_t), (iidx_d, iidx_t)):
                    nc.sync.dma_start(out=dst_t[0:16, c0:c1],
                                      in_=src_d.ap()[:, c0:c1])
                    nc.scalar.dma_start(out=dst_t[16:32, c0:c1],
                                        in_=src_d.ap()[:, c0:c1])
                    # One HWDGE wait (max tick of the two loads) lands on the
                    # first copy; the second and the gathers ride the DVE
                    # clock.
                    nc.vector.tensor_copy(out=dst_t[32:64, c0:c1],
                                          in_=dst_t[0:32, c0:c1])
                    nc.vector.tensor_copy(out=dst_t[64:128, c0:c1],
                                          in_=dst_t[0:64, c0:c1])

            scratch = scrp.tile([128, 512], F32)
            # PE warmup: observe the weight-load (HWDGE) semaphore.
            nc.tensor.matmul(out=scratch[0:64, 0:64], lhsT=w1_t[:],
                             rhs=w1_t[:], start=True, stop=True)

            nreg = {}
            for (_, s, _, _) in chunks:
                if s not in nreg:
                    nreg[s] = nc.gpsimd.to_reg(s * SUB)

            ubase = [g // 2 * TCH for g in range(8)]
            usize = [min(TCH, n_users - b) for b in ubase]
            ibase = [g % 2 * TCH for g in range(8)]
            isize = [min(TCH, n_items - b) for b in ibase]

            rt_tiles = [None] * T
            gu_tiles = [None] * T
            gi_tiles = [None] * T
            ps_tiles = [None] * T
            o_tiles = [None] * T

            last_g = [-1]

            def issue_loads(t):
                g, s, row, col = chunks[t]
                if g != last_g[0]:
                    fill_idx(g)
                    last_g[0] = g
                rt_t = rtp.tile([64, MAX_S * 128], BF16, tag="rt")
                nc.sync.dma_start(
                    out=rt_t[:, :s * 128],
                    in_=rt_d.ap()[:, row: row + s * 128])
                gu_t = gup.tile([128, MAX_S * DPAD], BF16, tag="gu")
                nc.gpsimd.dma_gather(
                    out_ap=gu_t[:, :s * DPAD].rearrange(
                        "p (n d) -> p n d", d=DPAD),
                    in_ap=up_d.ap()[ubase[g]:ubase[g] + usize[g], :],
                    idxs_ap=uidx_t[:, col:col + s * 8],
                    num_idxs=s * SUB, num_idxs_reg=nreg[s], elem_size=DPAD)
                gi_t = gip.tile([128, MAX_S * DPAD], BF16, tag="gi")
                nc.gpsimd.dma_gather(
                    out_ap=gi_t[:, :s * DPAD].rearrange(
                        "p (n d) -> p n d", d=DPAD),
                    in_ap=ip_d.ap()[ibase[g]:ibase[g] + isize[g], :],
                    idxs_ap=iidx_t[:, col:col + s * 8],
                    num_idxs=s * SUB, num_idxs_reg=nreg[s], elem_size=DPAD)
                rt_tiles[t], gu_tiles[t], gi_tiles[t] = rt_t, gu_t, gi_t

            def issue_matmuls(t):
                _, s, _, _ = chunks[t]
                rt_t = rt_tiles[t]
                ps = mmp.tile([128, MAX_S * 64], F32, tag="mm")
                # 1-column dummy absorbs the PSUM-recycle wait so the first
                # real matmul carries only the rt-load wait.
                nc.tensor.matmul(out=ps[0:64, 0:1], lhsT=w1_t[:],
                                 rhs=w1_t[:, 0:1], start=True, stop=True)
                for j in range(s):
                    nc.tensor.matmul(
                        out=ps[:, j * 64:(j + 1) * 64],
                        lhsT=rt_t[:, j * 128:(j + 1) * 128],
                        rhs=w1_t[:], start=True, stop=True)
                ps_tiles[t] = ps

            def issue_elemwise(t):
                _, s, _, _ = chunks[t]
                n = s * 64
                gu_t, gi_t = gu_tiles[t], gi_tiles[t]
                ps = ps_tiles[t]
                # Observer: absorb the gather-completion wait so the first
                # add keeps a single wait slot.
                nc.vector.tensor_copy(out=dscr[:, :], in_=gu_t[0:16, 0:16])
                t_t = ttp.tile([128, MAX_S * 64], BF16, tag="t")
                gu3 = gu_t[:, :s * DPAD].rearrange("p (n d) -> p n d", d=DPAD)
                gi3 = gi_t[:, :s * DPAD].rearrange("p (n d) -> p n d", d=DPAD)
                t3 = t_t[:, :n].rearrange("p (n d) -> p n d", d=64)
                nc.vector.scalar_tensor_tensor(
                    out=t3, in0=gu3[:, :, 0:64], scalar=0.0,
                    in1=gi3[:, :, 0:64], op0=BYP, op1=ADD)
                # psum += t  (in-place on PSUM; waits only on the last matmul)
                nc.vector.scalar_tensor_tensor(
                    out=ps[:, :n], in0=ps[:, :n], scalar=0.0,
                    in1=t_t[:, :n], op0=BYP, op1=ADD)

            def issue_relu(t):
                _, s, row, _ = chunks[t]
                n = s * 64
                o_t = outp.tile([128, MAX_S * 64], BF16, tag="o")
                # Observer: absorb the o_t store-recycle wait.
                nc.scalar.activation(out=ascr[:, :], in_=ascr[:, :],
                                     func=RELU)
                nc.scalar.activation(out=o_t[:, :n], in_=ps_tiles[t][:, :n],
                                     func=RELU)
                o_tiles[t] = o_t

            def issue_store(t):
                # Issued from the Activation HWDGE queue: the wait on the
                # relu is same-engine there, so SP's sequencer never blocks
                # ahead of the next chunk's loads.
                _, s, row, _ = chunks[t]
                nc.scalar.dma_start(
                    out=out_d.ap()[:, row // 2: row // 2 + s * 64],
                    in_=o_tiles[t][:, :s * 64])

            # Software-pipelined emission (see module docstring).
            for tt in range(min(PREF, T)):
                issue_loads(tt)
            for t in range(T):
                if t + PREF < T:
                    issue_loads(t + PREF)
                issue_matmuls(t)
                issue_elemwise(t)
                issue_relu(t)
                issue_store(t)
    nc.finalize()
    return nc


_PROGRAM_CACHE: dict = {}


def _get_program(chunk_key, n_users, n_items):
    key = (chunk_key, n_users, n_items)
    if key not in _PROGRAM_CACHE:
        _PROGRAM_CACHE[key] = (
            _build_program(_chunk_list(list(chunk_key)), n_users, n_items))
    return _PROGRAM_CACHE[key]


def _wrap_idx(flat_sorted, chunks):
    """Rebased int16 indices [PADTOT] -> [16, PADTOT//16] in dma_gather's
    wrapped layout: per chunk block [16, 8*s] with block[p, m] =
    flat[m*16 + p]; the device replicates across the 8 16-partition
    groups."""
    cols = []
    for (_, s, row, _) in chunks:
        blk = flat_sorted[row:row + s * SUB].reshape(s * 8, 16).T  # [16, 8s]
        cols.append(blk)
    return np.ascontiguousarray(np.concatenate(cols, axis=1))


def _run(review_vecs, user_vecs, item_vecs, W,
         review_user_adj, review_item_adj, perm_u, perm_i,
         n_cores, rpc):
    n_users = user_vecs.shape[0]
    n_items = item_vecs.shape[0]

    W = np.asarray(W, np.float32)
    W1 = np.ascontiguousarray(W[0:64])
    W2 = W[64:128]
    W3 = W[128:192]
    perm_u = np.asarray(perm_u, np.int64)
    perm_i = np.asarray(perm_i, np.int64)
    W2p = np.empty_like(W2)
    W2p[perm_u] = W2
    W3p = np.empty_like(W3)
    W3p[perm_i] = W3

    user_vecs = np.asarray(user_vecs, np.float32)
    item_vecs = np.asarray(item_vecs, np.float32)
    # Fold W2p/W3p into the tables; pad rows to 128 elems (256B in bf16).
    UP = np.zeros((n_users, DPAD), NP_BF16)
    UP[:, :64] = (user_vecs @ W2p).astype(NP_BF16)
    IP = np.zeros((n_items, DPAD), NP_BF16)
    IP[:, :64] = (item_vecs @ W3p).astype(NP_BF16)
    W1b = np.ascontiguousarray(W1.astype(NP_BF16))

    review_vecs = np.asarray(review_vecs, np.float32)
    au_all = np.asarray(review_user_adj, np.int64)
    ai_all = np.asarray(review_item_adj, np.int64)

    # Group ALL reviews by (user 32K chunk, item 32K chunk) and split each
    # group's reviews evenly across the cores: every core runs the same
    # chunk structure with minimal padding.
    grp_all = (au_all // TCH) * 2 + (ai_all // TCH)
    order_all = np.argsort(grp_all, kind="stable")
    counts_all = np.bincount(grp_all, minlength=8)
    gstart = np.concatenate([[0], np.cumsum(counts_all)])
    s_per_group = []
    for ctot in counts_all:
        per_core_max = -(-int(ctot) // n_cores)
        s_per_group.append(-(-per_core_max // SUB))
    chunk_key = tuple(s_per_group)
    chunks = _chunk_list(s_per_group)
    padtot = sum(s for (_, s, _, _) in chunks) * SUB

    nc = _get_program(chunk_key, n_users, n_items)

    in_maps = []
    slotmaps = []
    for c in range(n_cores):
        slotmap = np.full(padtot, -1, np.int64)
        row = 0
        for g in range(8):
            tot = int(counts_all[g])
            base, rem = divmod(tot, n_cores)
            cnt = base + (1 if c < rem else 0)
            off = c * base + min(c, rem)
            ids = order_all[gstart[g] + off: gstart[g] + off + cnt]
            slotmap[row:row + cnt] = ids
            row += s_per_group[g] * SUB
        valid = slotmap >= 0
        sl = np.where(valid, slotmap, 0)

        rv_sorted = np.where(valid[:, None], review_vecs[sl], 0.0)
        slot_g = np.repeat(np.arange(8), np.array(s_per_group) * SUB)
        u_reb = np.where(valid, au_all[sl] - (slot_g // 2) * TCH,
                         0).astype(np.int16)
        i_reb = np.where(valid, ai_all[sl] - (slot_g % 2) * TCH,
                         0).astype(np.int16)

        in_maps.append({
            "rt": np.ascontiguousarray(rv_sorted.T.astype(NP_BF16)),
            "uidx": _wrap_idx(u_reb, chunks),
            "iidx": _wrap_idx(i_reb, chunks),
            "up": UP,
            "ip": IP,
            "w1": W1b,
        })
        slotmaps.append((slotmap, valid))

    res = run_bass_kernel_spmd(nc, in_maps, core_ids=list(range(n_cores)))

    out = np.empty((n_cores * rpc, 64), np.float32)
    for c in range(n_cores):
        o2 = np.asarray(res.results[c]["out2"])
        # [128, padtot//2] wrapped row-major -> [padtot, 64]
        out_sorted = np.ascontiguousarray(
            o2.reshape(128, padtot // 128, 64).transpose(1, 0, 2)
        ).reshape(padtot, 64).astype(np.float32)
        slotmap, valid = slotmaps[c]
        out[slotmap[valid]] = out_sorted[valid]
    return out


def kernel(**inputs) -> np.ndarray:
    return _run(
        inputs["review_vecs"], inputs["user_vecs"], inputs["item_vecs"],
        inputs["W"], inputs["review_user_adj"], inputs["review_item_adj"],
        inputs["perm_u"], inputs["perm_i"],
        n_cores=N_CORES, rpc=RPC)


# revision 29
# speedup vs baseline: 2.5152x; 1.3189x over previous
"""Trainium2 Bass kernel for nn_ConcatenationAggregator.

For each review r:
    out[r] = relu(concat(review_vecs[r],
                         user_vecs[adj_u[r]][perm_u],
                         item_vecs[adj_i[r]][perm_i]) @ W)

Strategy (pure data-parallel over reviews, 8 NeuronCores):
  - The feature permutations AND the W2/W3 blocks of W are folded into the
    tables on the host: UP = user_vecs @ W2p, IP = item_vecs @ W3p.  Then
    out[r] = relu(review_vecs[r] @ W1 + UP[adj_u[r]] + IP[adj_i[r]]), i.e.
    the gathered rows are pure elementwise addends and the only device
    matmul left is the review term.
  - Everything on device is bf16 except the f32 PSUM accumulator: review
    stream, projected tables (rows padded to 128 elems = 256B so dma_gather
    accepts them), and the stored output.  This halves HBM/DMA traffic for
    the streamed tensors.
  - Row gathers use the GPSIMD `dma_gather` ucode (int16 indices, <=1024
    indices per call).  Since the tables exceed 32768 rows, the host sorts
    each core's reviews into 8 groups by (user-table 32K chunk, item-table
    32K chunk) so that rebased indices fit int16; the host un-permutes the
    output (identical scheme to the index layout dma_gather wants).
  - Layout: reviews are processed in 1024-row chunks of 8 sub-tiles.  The
    review stream is host-transposed to feature-major [64, slots]; each
    sub-tile j is a [64, 128] stationary lhsT and W1 the [64, 64] moving
    rhs, producing a row-major [128 rows, 64] PSUM block per sub-tile
    (one PSUM bank per chunk).  Gathered rows arrive row-major from
    dma_gather, so the user+item contributions are two DVE adds and the
    relu runs on the otherwise-idle Activation engine; output is stored
    row-major-wrapped [128, slots*64/128] and unwrapped on the host.
  - This toolchain build enforces ONE sync-wait slot per instruction, so
    tiny "observer" ops absorb extra cross-engine waits: a 1-column PE
    matmul takes the PSUM-recycle wait, small DVE/Act copies take the
    gather-completion and buffer-recycle waits, and the kernel-tail drain
    is split into single-wait drains.
"""

import os
import types

import numpy as np
import ml_dtypes

import concourse.bacc as bacc
import concourse.mybir as mybir
import concourse.tile as tile
from concourse import ap_utils
from concourse.bass_utils import run_bass_kernel_spmd
from concourse.vector_clock import ScopedClock, VectorClock

F32 = mybir.dt.float32
BF16 = mybir.dt.bfloat16
I16 = mybir.dt.int16

NP_BF16 = ml_dtypes.bfloat16

N_CORES = 8
D = 64
DPAD = 128                 # padded table row (256B in bf16, dma_gather min)
SUB = 128                  # reviews per sub-tile
MAX_S = 8                  # sub-tiles per chunk (<=1024 gather indices)
TCH = 32768                # table chunk (int16 index range)

N_REVIEWS = 1_000_000
N_USERS = 100_000
N_ITEMS = 50_000
RPC = N_REVIEWS // N_CORES

BUFS = int(os.environ.get("KBUFS", "5"))
PREF = int(os.environ.get("KPREF", "4"))


def _split_drain_and_barrier(self, tick_clock, wait_clock):
    """Replacement for TileContext._drain_and_barrier: the stock tail drain
    waits on every live proc semaphore at once, which overflows this
    toolchain's one-sync-wait-per-instruction limit.  Emit one drain per
    semaphore instead."""
    gc = tick_clock.global_clock
    ticks = list(gc)
    idxs = [i for i, t in enumerate(ticks) if t > 0]
    for i in idxs:
        sub = [0] * len(ticks)
        sub[i] = ticks[i]
        drain_inst = self.nc.sync.drain()
        wait_clock.add_sem_waits(
            drain_inst.ins, ScopedClock({None: VectorClock(sub)}))
    if not idxs:
        drain_inst = self.nc.sync.drain()
        wait_clock.add_sem_waits(
            drain_inst.ins, ScopedClock({None: VectorClock(ticks)}))
    self.nc.all_engine_barrier()
    assert self.sems is not None
    popped = self.nc._tile_sem_poison_stack.pop()
    assert popped is self._sem_poison
    self.nc.clear_and_free_semaphores(list(self.sems.allocated().values()))
    self.nc.all_engine_barrier()


def _raw_dma_gather(gp, out_ap, in_ap, idxs_ap, num_idxs, num_idxs_reg,
                    elem_size, elem_step):
    """BassGpSimd.dma_gather without the elem_size_bytes%256 assert: the
    gather ucode strides in 256B quanta (stride_bytes_256) but transfers
    elem_size bytes per descriptor, so a 256B-stride table with 128B live
    rows gathers at half the descriptor cost.  Verified bit-exact on
    hardware (idx addresses the 256B-stride row, descriptors carry the
    first 128B)."""
    assert idxs_ap.dtype == mybir.dt.int16
    assert in_ap.dtype == out_ap.dtype
    assert ap_utils.ap_is_contiguous(in_ap.ap[1:])
    assert ap_utils.ap_is_contiguous(out_ap.ap[1:])
    assert ap_utils.ap_is_contiguous(idxs_ap.ap[1:])
    assert in_ap.ap[0][0] == elem_step
    assert num_idxs % 128 == 0
    assert out_ap.ap[-1][1] == elem_size
    stride_bytes = elem_step * mybir.dt.size(in_ap.dtype)
    stride_bytes_256 = stride_bytes // 256
    assert stride_bytes % 256 == 0 and 0 < stride_bytes_256 < 256
    _in_ap = gp.lower_ap_dma(in_ap, for_custom_bir_dma=True)
    _idxs_ap = gp.lower_ap(idxs_ap)
    _out_ap = gp.lower_ap(out_ap)
    return gp.add_instruction(
        mybir.InstDMAGatherAnt(
            name=gp.bass.get_next_instruction_name(),
            ins=[*_in_ap, _idxs_ap,
                 gp.lower_val_access(gp.to_reg(num_idxs_reg))],
            outs=[_out_ap],
            transpose=False,
            num_idxs=num_idxs,
            elem_size=elem_size,
            stride_bytes_256=stride_bytes_256,
            gen_mode=0,
            single_packet=True,
            queue_num=0,
            sbuf_tokens_per_rank=0,
            sbuf_free_dim_per_rank=0,
            sbuf_free_dim_pad_per_rank=0,
            sbuf_byte_offset=0,
        ))


def _chunk_list(s_per_group):
    """[(group, s_subtiles, row_base_slots, idxcol_base), ...] — shared by
    host packing and device program.  s values are <= MAX_S."""
    chunks = []
    row = 0
    col = 0
    for g, sg in enumerate(s_per_group):
        left = sg
        while left > 0:
            s = min(MAX_S, left)
            chunks.append((g, s, row, col))
            row += s * SUB
            col += s * 8
            left -= s
    return chunks


def _build_program(chunks, n_users, n_items):
    nc = bacc.Bacc("TRN2", target_bir_lowering=False, debug=False,
                   enable_asserts=False)
    padtot = sum(s for (_, s, _, _) in chunks) * SUB
    icols = padtot // 16

    rt_d = nc.dram_tensor("rt", [64, padtot], BF16, kind="ExternalInput")
    uidx_d = nc.dram_tensor("uidx", [16, icols], I16, kind="ExternalInput")
    iidx_d = nc.dram_tensor("iidx", [16, icols], I16, kind="ExternalInput")
    up_d = nc.dram_tensor("up", [n_users, DPAD], BF16, kind="ExternalInput")
    ip_d = nc.dram_tensor("ip", [n_items, DPAD], BF16, kind="ExternalInput")
    w1_d = nc.dram_tensor("w1", [64, 64], BF16, kind="ExternalInput")
    out_d = nc.dram_tensor("out2", [128, padtot // 2], BF16,
                           kind="ExternalOutput")

    T = len(chunks)
    RELU = mybir.ActivationFunctionType.Relu
    BYP = mybir.AluOpType.bypass
    ADD = mybir.AluOpType.add

    with tile.TileContext(nc) as tc:
        tc._drain_and_barrier = types.MethodType(_split_drain_and_barrier, tc)
        with tc.tile_pool(name="const", bufs=1) as constp, \
             tc.tile_pool(name="rtp", bufs=BUFS) as rtp, \
             tc.tile_pool(name="gup", bufs=BUFS) as gup, \
             tc.tile_pool(name="gip", bufs=BUFS) as gip, \
             tc.tile_pool(name="ttp", bufs=BUFS) as ttp, \
             tc.tile_pool(name="outp", bufs=BUFS) as outp, \
             tc.tile_pool(name="scr", bufs=1, space="PSUM") as scrp, \
             tc.tile_pool(name="mmp", bufs=BUFS, space="PSUM") as mmp:

            w1_t = constp.tile([64, 64], BF16)
            nc.sync.dma_start(out=w1_t[:], in_=w1_d.ap()[:, :])
            # Index tables: the gather ucode wants the 16-partition-wrapped
            # indices replicated across all 8 GPSIMD cores.  Only the unique
            # 16 partitions travel over DMA (per-group slices, so the first
            # gathers don't stall on the full upload); the 8x replication
            # runs on the lightly-loaded DVE engine.  Engine APs may only
            # start at partition 0/32/64/96, so DMA fills [0:16] and [16:32]
            # and DVE doubles [0:32]->[32:64] and [0:64]->[64:128].
            uidx_t = constp.tile([128, icols], I16)
            iidx_t = constp.tile([128, icols], I16)
            gcols: dict = {}
            for (g, s, _, col) in chunks:
                c0, c1 = gcols.get(g, (col, col))
                gcols[g] = (min(c0, col), max(c1, col + s * 8))
            dscr = constp.tile([16, 16], BF16)
            ascr = constp.tile([16, 16], BF16)

            def fill_idx(g):
                c0, c1 = gcols[g]
                for src_d, dst_t in ((uidx_d, uidx_t), (iidx_d, iidx_t)):
                    nc.sync.dma_start(out=dst_t[0:16, c0:c1],
                                      in_=src_d.ap()[:, c0:c1])
                    nc.scalar.dma_start(out=dst_t[16:32, c0:c1],
                                        in_=src_d.ap()[:, c0:c1])
                    # One HWDGE wait (max tick of the two loads) lands on the
                    # first copy; the second and the gathers ride the DVE
                    # clock.
                    nc.vector.tensor_copy(out=dst_t[32:64, c0:c1],
                                          in_=dst_t[0:32, c0:c1])
                    nc.vector.tensor_copy(out=dst_t[64:128, c0:c1],
                                          in_=dst_t[0:64, c0:c1])

            scratch = scrp.tile([128, 512], F32)
            # PE warmup: observe the weight-load (HWDGE) semaphore.
            nc.tensor.matmul(out=scratch[0:64, 0:64], lhsT=w1_t[:],
                             rhs=w1_t[:], start=True, stop=True)

            nreg = {}
            for (_, s, _, _) in chunks:
                if s not in nreg:
                    nreg[s] = nc.gpsimd.to_reg(s * SUB)

            ubase = [g // 2 * TCH for g in range(8)]
            usize = [min(TCH, n_users - b) for b in ubase]
            ibase = [g % 2 * TCH for g in range(8)]
            isize = [min(TCH, n_items - b) for b in ibase]

            rt_tiles = [None] * T
            gu_tiles = [None] * T
            gi_tiles = [None] * T
            ps_tiles = [None] * T
            o_tiles = [None] * T

            last_g = [-1]

            def issue_loads(t):
                g, s, row, col = chunks[t]
                if g != last_g[0]:
                    fill_idx(g)
                    last_g[0] = g
                rt_t = rtp.tile([64, MAX_S * 128], BF16, tag="rt")
                nc.sync.dma_start(
                    out=rt_t[:, :s * 128],
                    in_=rt_d.ap()[:, row: row + s * 128])
                gu_t = gup.tile([128, MAX_S * 64], BF16, tag="gu")
                _raw_dma_gather(
                    nc.gpsimd,
                    out_ap=gu_t[:, :s * 64].rearrange(
                        "p (n d) -> p n d", d=64),
                    in_ap=up_d.ap()[ubase[g]:ubase[g] + usize[g], :],
                    idxs_ap=uidx_t[:, col:col + s * 8],
                    num_idxs=s * SUB, num_idxs_reg=nreg[s],
                    elem_size=64, elem_step=DPAD)
                gi_t = gip.tile([128, MAX_S * 64], BF16, tag="gi")
                _raw_dma_gather(
                    nc.gpsimd,
                    out_ap=gi_t[:, :s * 64].rearrange(
                        "p (n d) -> p n d", d=64),
                    in_ap=ip_d.ap()[ibase[g]:ibase[g] + isize[g], :],
                    idxs_ap=iidx_t[:, col:col + s * 8],
                    num_idxs=s * SUB, num_idxs_reg=nreg[s],
                    elem_size=64, elem_step=DPAD)
                rt_tiles[t], gu_tiles[t], gi_tiles[t] = rt_t, gu_t, gi_t

            def issue_matmuls(t):
                _, s, _, _ = chunks[t]
                rt_t = rt_tiles[t]
                ps = mmp.tile([128, MAX_S * 64], F32, tag="mm")
                # 1-column dummy absorbs the PSUM-recycle wait so the first
                # real matmul carries only the rt-load wait.
                nc.tensor.matmul(out=ps[0:64, 0:1], lhsT=w1_t[:],
                                 rhs=w1_t[:, 0:1], start=True, stop=True)
                for j in range(s):
                    nc.tensor.matmul(
                        out=ps[:, j * 64:(j + 1) * 64],
                        lhsT=rt_t[:, j * 128:(j + 1) * 128],
                        rhs=w1_t[:], start=True, stop=True)
                ps_tiles[t] = ps

            def issue_elemwise(t):
                _, s, _, _ = chunks[t]
                n = s * 64
                gu_t, gi_t = gu_tiles[t], gi_tiles[t]
                ps = ps_tiles[t]
                # Observer: absorb the gather-completion wait so the first
                # add keeps a single wait slot.
                nc.vector.tensor_copy(out=dscr[:, :], in_=gu_t[0:16, 0:16])
                t_t = ttp.tile([128, MAX_S * 64], BF16, tag="t")
                nc.vector.scalar_tensor_tensor(
                    out=t_t[:, :n], in0=gu_t[:, :n], scalar=0.0,
                    in1=gi_t[:, :n], op0=BYP, op1=ADD)
                # psum += t  (in-place on PSUM; waits only on the last matmul)
                nc.vector.scalar_tensor_tensor(
                    out=ps[:, :n], in0=ps[:, :n], scalar=0.0,
                    in1=t_t[:, :n], op0=BYP, op1=ADD)

            def issue_relu(t):
                _, s, row, _ = chunks[t]
                n = s * 64
                o_t = outp.tile([128, MAX_S * 64], BF16, tag="o")
                # Observer: absorb the o_t store-recycle wait.
                nc.scalar.activation(out=ascr[:, :], in_=ascr[:, :],
                                     func=RELU)
                nc.scalar.activation(out=o_t[:, :n], in_=ps_tiles[t][:, :n],
                                     func=RELU)
                o_tiles[t] = o_t

            def issue_store(t):
                # Issued from the Activation HWDGE queue: the wait on the
                # relu is same-engine there, so SP's sequencer never blocks
                # ahead of the next chunk's loads.
                _, s, row, _ = chunks[t]
                nc.scalar.dma_start(
                    out=out_d.ap()[:, row // 2: row // 2 + s * 64],
                    in_=o_tiles[t][:, :s * 64])

            # Software-pipelined emission (see module docstring).
            for tt in range(min(PREF, T)):
                issue_loads(tt)
            for t in range(T):
                if t + PREF < T:
                    issue_loads(t + PREF)
                issue_matmuls(t)
                issue_elemwise(t)
                issue_relu(t)
                issue_store(t)
    nc.finalize()
    return nc


_PROGRAM_CACHE: dict = {}


def _get_program(chunk_key, n_users, n_items):
    key = (chunk_key, n_users, n_items)
    if key not in _PROGRAM_CACHE:
        _PROGRAM_CACHE[key] = (
            _build_program(_chunk_list(list(chunk_key)), n_users, n_items))
    return _PROGRAM_CACHE[key]


def _wrap_idx(flat_sorted, chunks):
    """Rebased int16 indices [PADTOT] -> [16, PADTOT//16] in dma_gather's
    wrapped layout: per chunk block [16, 8*s] with block[p, m] =
    flat[m*16 + p]; the device replicates across the 8 16-partition
    groups."""
    cols = []
    for (_, s, row, _) in chunks:
        blk = flat_sorted[row:row + s * SUB].reshape(s * 8, 16).T  # [16, 8s]
        cols.append(blk)
    return np.ascontiguousarray(np.concatenate(cols, axis=1))


def _run(review_vecs, user_vecs, item_vecs, W,
         review_user_adj, review_item_adj, perm_u, perm_i,
         n_cores, rpc):
    n_users = user_vecs.shape[0]
    n_items = item_vecs.shape[0]

    W = np.asarray(W, np.float32)
    W1 = np.ascontiguousarray(W[0:64])
    W2 = W[64:128]
    W3 = W[128:192]
    perm_u = np.asarray(perm_u, np.int64)
    perm_i = np.asarray(perm_i, np.int64)
    W2p = np.empty_like(W2)
    W2p[perm_u] = W2
    W3p = np.empty_like(W3)
    W3p[perm_i] = W3

    user_vecs = np.asarray(user_vecs, np.float32)
    item_vecs = np.asarray(item_vecs, np.float32)
    # Fold W2p/W3p into the tables; pad rows to 128 elems (256B in bf16).
    UP = np.zeros((n_users, DPAD), NP_BF16)
    UP[:, :64] = (user_vecs @ W2p).astype(NP_BF16)
    IP = np.zeros((n_items, DPAD), NP_BF16)
    IP[:, :64] = (item_vecs @ W3p).astype(NP_BF16)
    W1b = np.ascontiguousarray(W1.astype(NP_BF16))

    review_vecs = np.asarray(review_vecs, np.float32)
    au_all = np.asarray(review_user_adj, np.int64)
    ai_all = np.asarray(review_item_adj, np.int64)

    # Group ALL reviews by (user 32K chunk, item 32K chunk) and split each
    # group's reviews evenly across the cores: every core runs the same
    # chunk structure with minimal padding.
    grp_all = (au_all // TCH) * 2 + (ai_all // TCH)
    order_all = np.argsort(grp_all, kind="stable")
    counts_all = np.bincount(grp_all, minlength=8)
    gstart = np.concatenate([[0], np.cumsum(counts_all)])
    s_per_group = []
    for ctot in counts_all:
        per_core_max = -(-int(ctot) // n_cores)
        s_per_group.append(-(-per_core_max // SUB))
    chunk_key = tuple(s_per_group)
    chunks = _chunk_list(s_per_group)
    padtot = sum(s for (_, s, _, _) in chunks) * SUB

    nc = _get_program(chunk_key, n_users, n_items)

    in_maps = []
    slotmaps = []
    for c in range(n_cores):
        slotmap = np.full(padtot, -1, np.int64)
        row = 0
        for g in range(8):
            tot = int(counts_all[g])
            base, rem = divmod(tot, n_cores)
            cnt = base + (1 if c < rem else 0)
            off = c * base + min(c, rem)
            ids = order_all[gstart[g] + off: gstart[g] + off + cnt]
            slotmap[row:row + cnt] = ids
            row += s_per_group[g] * SUB
        valid = slotmap >= 0
        sl = np.where(valid, slotmap, 0)

        rv_sorted = np.where(valid[:, None], review_vecs[sl], 0.0)
        slot_g = np.repeat(np.arange(8), np.array(s_per_group) * SUB)
        u_reb = np.where(valid, au_all[sl] - (slot_g // 2) * TCH,
                         0).astype(np.int16)
        i_reb = np.where(valid, ai_all[sl] - (slot_g % 2) * TCH,
                         0).astype(np.int16)

        in_maps.append({
            "rt": np.ascontiguousarray(rv_sorted.T.astype(NP_BF16)),
            "uidx": _wrap_idx(u_reb, chunks),
            "iidx": _wrap_idx(i_reb, chunks),
            "up": UP,
            "ip": IP,
            "w1": W1b,
        })
        slotmaps.append((slotmap, valid))

    res = run_bass_kernel_spmd(nc, in_maps, core_ids=list(range(n_cores)))

    out = np.empty((n_cores * rpc, 64), np.float32)
    for c in range(n_cores):
        o2 = np.asarray(res.results[c]["out2"])
        # [128, padtot//2] wrapped row-major -> [padtot, 64]
        out_sorted = np.ascontiguousarray(
            o2.reshape(128, padtot // 128, 64).transpose(1, 0, 2)
        ).reshape(padtot, 64).astype(np.float32)
        slotmap, valid = slotmaps[c]
        out[slotmap[valid]] = out_sorted[valid]
    return out


def kernel(**inputs) -> np.ndarray:
    return _run(
        inputs["review_vecs"], inputs["user_vecs"], inputs["item_vecs"],
        inputs["W"], inputs["review_user_adj"], inputs["review_item_adj"],
        inputs["perm_u"], inputs["perm_i"],
        n_cores=N_CORES, rpc=RPC)
